# revision 1
# baseline (speedup 1.0000x reference)
"""ConvLSTM decoder (2 ConvLSTM layers + top conv) on 8 Trainium2 cores.

Sharding: data-parallel over batch — B=8, one batch element per core,
weights replicated. The T=10 recurrence runs fully on-core.

Layout: images are stored in SBUF as a zero-padded flat row-major strip:
each 64-pixel row padded to 66 cols (1 zero col each side), 64 rows
contiguous, plus 68-col zero margins at both ends. A 3x3 'SAME' conv then
becomes 9 shifted matmuls accumulated in PSUM: for tap (dy,dx) the rhs is
the flat strip shifted by dy*66+dx.

Layer-0 i2h has only 64 input channels (half the PE array). Its 9 taps are
packed into 5 matmul groups of K=128 by stacking x with a shifted copy of
x in partitions 64:128 of two buffers (shift +66 pairs taps {-67,-1},
{-66,0},{-65,1}; shift +2 pairs {65,67}; tap 66 stays K=64).
"""

import numpy as np

B, T, C, H, W = 8, 10, 64, 64, 64
CH = 128
NSTEP = T - 1          # 9 recurrent steps
WP = W + 2             # padded row width
FLAT = H * WP          # 4224
MARG = 68              # >= 67 = max |tap offset|
BUFC = MARG + FLAT + MARG
BASE = MARG
HW = H * W             # 4096

# row chunks (r0, r1): 8x7 rows + 2x4 rows; max matmul N = 7*66 = 462 <= 512
CHUNKS = [(i * 7, i * 7 + 7) for i in range(8)] + [(56, 60), (60, 64)]
# weight-sharing groups: matmuls per ldweights = group size; <=4 banks open
CGROUPS = [(0, 4), (4, 8), (8, 10)]

TAPS = [(dy, dx) for dy in (-1, 0, 1) for dx in (-1, 0, 1)]

# layer-0 i2h tap packing: (offset_of_group0_tap, paired?) per slot;
# slots 0-2 pair (o, o+66) on xb66; slot 3 pairs (65, 67) on xb2;
# slot 4 is the lone K=64 tap at offset 66 read from xb66[0:64].
L0SLOTS = [(-67, "xb66", True), (-66, "xb66", True), (-65, "xb66", True),
           (65, "xb2", True), (66, "xb66", False)]
# (ky,kx) kernel indices per slot: group0 tap, group1 tap
L0SLOT_KK = [((0, 0), (1, 0)), ((0, 1), (1, 1)), ((0, 2), (1, 2)),
             ((2, 0), (2, 2)), ((2, 1), None)]

MM_DT = "bf16"         # "f32" | "f32r" | "bf16"
LOOP_N = 0             # >0: wrap body in a hardware repeat loop (timing only)

_CACHE = {}


def _np_dt(mybir):
    if MM_DT == "bf16":
        return mybir.dt.bfloat16
    if MM_DT == "f32r":
        return mybir.dt.float32r
    return mybir.dt.float32


def _host_cast(a):
    if MM_DT == "bf16":
        import ml_dtypes
        return np.ascontiguousarray(a.astype(ml_dtypes.bfloat16))
    return np.ascontiguousarray(a.astype(np.float32))


def _prep_w(w):
    # [O, I, 3, 3] -> [I, 9*O]; slice for (tap ti, 128-chunk g): ti*O + g*128
    O, I = w.shape[0], w.shape[1]
    return _host_cast(w.transpose(1, 2, 3, 0).reshape(I, 9 * O))


def _prep_w0(w):
    # [512, 64, 3, 3] -> [128, 5*512] slot-stacked for L0 i2h packing
    O, I = w.shape[0], w.shape[1]
    out = np.zeros((2 * I, 5 * O), np.float32)
    for k, (a, b) in enumerate(L0SLOT_KK):
        out[:I, k * O:(k + 1) * O] = w[:, :, a[0], a[1]].T
        if b is not None:
            out[I:2 * I, k * O:(k + 1) * O] = w[:, :, b[0], b[1]].T
    return _host_cast(out)


def _build():
    import concourse.bass as bass
    import concourse.tile as tile
    from concourse import bacc, mybir

    f32 = mybir.dt.float32
    cdt = _np_dt(mybir)          # matmul-input dtype in SBUF (and DRAM)
    AF = mybir.ActivationFunctionType

    nc = bacc.Bacc("TRN2", target_bir_lowering=False, debug=False,
                   num_devices=8)

    ddt = cdt if MM_DT in ("f32r", "bf16") else f32
    xs_d = nc.dram_tensor("xs", [NSTEP, C, HW], ddt, kind="ExternalInput")
    h0_d = nc.dram_tensor("h0i", [CH, HW], ddt, kind="ExternalInput")
    c0_d = nc.dram_tensor("c0i", [CH, HW], f32, kind="ExternalInput")
    h1_d = nc.dram_tensor("h1i", [CH, HW], ddt, kind="ExternalInput")
    c1_d = nc.dram_tensor("c1i", [CH, HW], f32, kind="ExternalInput")
    w0_d = nc.dram_tensor("w0", [2 * C, 5 * 4 * CH], ddt, kind="ExternalInput")
    u0_d = nc.dram_tensor("u0", [CH, 9 * 4 * CH], ddt, kind="ExternalInput")
    w1_d = nc.dram_tensor("w1", [CH, 9 * 4 * CH], ddt, kind="ExternalInput")
    u1_d = nc.dram_tensor("u1", [CH, 9 * 4 * CH], ddt, kind="ExternalInput")
    wt_d = nc.dram_tensor("wt", [CH, 9 * C], ddt, kind="ExternalInput")
    zz_d = nc.dram_tensor("zz", [CH, BUFC], ddt, kind="ExternalInput")
    b0_d = nc.dram_tensor("b0", [CH, 4], f32, kind="ExternalInput")
    b1_d = nc.dram_tensor("b1", [CH, 4], f32, kind="ExternalInput")
    bt_d = nc.dram_tensor("bt", [C, 1], f32, kind="ExternalInput")
    out_d = nc.dram_tensor("out", [T, C, HW], f32, kind="ExternalOutput")

    def interior(ap_2d, s0, nrow):
        # rows of 64 interior cols at stride 66 starting at flat offset s0
        return ap_2d[:, s0:s0 + nrow * WP].rearrange(
            "p (r c) -> p r c", c=WP)[:, :, 1:1 + W]

    with tile.TileContext(nc) as tc:
        with (
            tc.tile_pool(name="pers", bufs=1) as pers,
            tc.tile_pool(name="ps", bufs=8, space="PSUM") as psp,
            tc.tile_pool(name="gt", bufs=5) as gtp,
            tc.tile_pool(name="osb", bufs=5) as osbp,
        ):
            # --- persistent SBUF residents ---
            w0_t = pers.tile([2 * C, 5 * 4 * CH], cdt, tag="w0")
            u0_t = pers.tile([CH, 9 * 4 * CH], cdt, tag="u0")
            w1_t = pers.tile([CH, 9 * 4 * CH], cdt, tag="w1")
            u1_t = pers.tile([CH, 9 * 4 * CH], cdt, tag="u1")
            wt_t = pers.tile([CH, 9 * C], cdt, tag="wt")
            b0_t = pers.tile([CH, 4], f32, tag="b0")
            b1_t = pers.tile([CH, 4], f32, tag="b1")
            bt_t = pers.tile([C, 1], f32, tag="bt")
            xb66 = pers.tile([2 * C, BUFC], cdt, tag="xb66")
            xb2 = pers.tile([2 * C, BUFC], cdt, tag="xb2")
            h0p = [pers.tile([CH, BUFC], cdt, tag=f"h0p{i}", name=f"h0p{i}")
                   for i in range(2)]
            h1p = [pers.tile([CH, BUFC], cdt, tag=f"h1p{i}", name=f"h1p{i}")
                   for i in range(2)]
            c0_t = pers.tile([CH, HW], f32, tag="c0")
            c1_t = pers.tile([CH, HW], f32, tag="c1")

            for t_, d_ in ((w0_t, w0_d), (u0_t, u0_d), (w1_t, w1_d),
                           (u1_t, u1_d), (wt_t, wt_d), (b0_t, b0_d),
                           (b1_t, b1_d), (bt_t, bt_d)):
                nc.sync.dma_start(t_[:], d_.ap())

            # one-time zero fill (margins/padding stay zero forever; the
            # interiors are fully re-written by DMA/compute every iteration)
            for buf in (xb66, xb2, h0p[0], h0p[1], h1p[0], h1p[1]):
                if MM_DT == "f32r":
                    nc.sync.dma_start(buf[:], zz_d.ap()[:buf.shape[0]])
                else:
                    nc.vector.memset(buf[:], 0.0)

            def init_states():
                nc.sync.dma_start(interior(h1p[0], BASE, H), h1_d.ap())
                load_x(0)
                nc.sync.dma_start(interior(h0p[0], BASE, H), h0_d.ap())
                nc.sync.dma_start(c0_t[:], c0_d.ap())
                nc.sync.dma_start(c1_t[:], c1_d.ap())

            def l0_xtaps(g):
                # x-side matmul slots for layer 0: (lhs, src, np_rhs, off)
                res = []
                for k, (off, srcn, paired) in enumerate(L0SLOTS):
                    src = xb66 if srcn == "xb66" else xb2
                    o = k * 4 * CH + g * CH
                    kk = 2 * C if paired else C
                    res.append((w0_t[:kk, o:o + CH], src, kk, off))
                return res

    # taps for a standard 9-tap conv operand
            def std_taps(w_t, src, kx, g):
                res = []
                for ti in range(9):
                    dy, dx = TAPS[ti]
                    o = ti * 4 * CH + g * CH
                    res.append((w_t[:kx, o:o + CH], src, kx, dy * WP + dx))
                return res

            def conv_gates(xtaps_fn, hin, wh_t, b_t, c_t, hout, h_first):
                """One ConvLSTM cell; chunk groups share stationary
                weights (group-size matmuls per ldweights)."""
                for bi, be in CGROUPS:
                    pair = CHUNKS[bi:be]
                    gtiles = [[None] * 4 for _ in pair]
                    for g in range(4):
                        pss = [psp.tile([CH, (r1 - r0) * WP], f32, tag="ps",
                                        name="ps") for (r0, r1) in pair]
                        xt = xtaps_fn(g)
                        ht = std_taps(wh_t, hin, CH, g)
                        taps = (ht + xt) if h_first else (xt + ht)
                        nt = len(taps)
                        for k, (lhs, src, kk, off) in enumerate(taps):
                            for j, (r0, r1) in enumerate(pair):
                                s = BASE + r0 * WP + off
                                cw = (r1 - r0) * WP
                                nc.tensor.matmul(pss[j][:], lhs,
                                                 src[:kk, s:s + cw],
                                                 start=(k == 0),
                                                 stop=(k == nt - 1))
                        for j, (r0, r1) in enumerate(pair):
                            nr = r1 - r0
                            gt = gtp.tile([CH, nr * W], f32, tag=f"g{g}",
                                          name=f"g{g}")
                            func = AF.Tanh if g == 2 else AF.Sigmoid
                            nc.scalar.activation(
                                gt[:].rearrange("p (r c) -> p r c", c=W),
                                pss[j][:].rearrange(
                                    "p (r c) -> p r c", c=WP)[:, :, 1:1 + W],
                                func, bias=b_t[:, g:g + 1])
                            gtiles[j][g] = gt
                    for j, (r0, r1) in enumerate(pair):
                        nr = r1 - r0
                        gi, gf, gg, go = gtiles[j]
                        csl = c_t[:, r0 * W:r1 * W]
                        nc.vector.tensor_mul(gg[:], gi[:], gg[:])   # i*g
                        nc.vector.tensor_mul(csl, gf[:], csl)       # f*c
                        nc.vector.tensor_add(csl, csl, gg[:])       # c
                        nc.scalar.activation(gf[:], csl, AF.Tanh)
                        nc.vector.tensor_mul(
                            interior(hout, BASE + r0 * WP, nr),
                            go[:].rearrange("p (r c) -> p r c", c=W),
                            gf[:].rearrange("p (r c) -> p r c", c=W))

            def conv_top(hin, tout):
                for bi, be in CGROUPS:
                    pair = CHUNKS[bi:be]
                    pss = [psp.tile([C, (r1 - r0) * WP], f32, tag="ps",
                                    name="ps") for (r0, r1) in pair]
                    for ti in range(9):
                        dy, dx = TAPS[ti]
                        lhs = wt_t[:, ti * C:(ti + 1) * C]
                        for j, (r0, r1) in enumerate(pair):
                            s = BASE + r0 * WP + dy * WP + dx
                            cw = (r1 - r0) * WP
                            nc.tensor.matmul(pss[j][:], lhs, hin[:, s:s + cw],
                                             start=(ti == 0), stop=(ti == 8))
                    for j, (r0, r1) in enumerate(pair):
                        nr = r1 - r0
                        ot = osbp.tile([C, nr * W], f32, tag="ot", name="ot")
                        nc.scalar.activation(
                            ot[:].rearrange("p (r c) -> p r c", c=W),
                            pss[j][:].rearrange(
                                "p (r c) -> p r c", c=WP)[:, :, 1:1 + W],
                            AF.Identity, bias=bt_t[:, 0:1])
                        nc.gpsimd.dma_start(tout[:, r0 * W:r1 * W], ot[:])

            def load_x(t):
                # x strip into: xb66[0:64]@BASE, xb66[64:128]@BASE-66,
                # xb2[0:64]@BASE, xb2[64:128]@BASE-2
                src = xs_d.ap()[t]
                nc.sync.dma_start(
                    interior(xb66[:C, :], BASE, H), src)
                nc.sync.dma_start(
                    interior(xb66[C:2 * C, :], BASE - 66, H), src)
                nc.sync.dma_start(
                    interior(xb2[:C, :], BASE, H), src)
                nc.sync.dma_start(
                    interior(xb2[C:2 * C, :], BASE - 2, H), src)

            def l1_xtaps_for(h0buf):
                return lambda g: std_taps(w1_t, h0buf, CH, g)

            def body():
                init_states()
                conv_top(h1p[0], out_d.ap()[0])
                for t in range(NSTEP):
                    if t > 0:
                        load_x(t)
                    conv_gates(l0_xtaps, h0p[t % 2], u0_t, b0_t, c0_t,
                               h0p[(t + 1) % 2], h_first=False)
                    conv_gates(l1_xtaps_for(h0p[(t + 1) % 2]), h1p[t % 2],
                               u1_t, b1_t, c1_t, h1p[(t + 1) % 2],
                               h_first=True)
                    conv_top(h1p[(t + 1) % 2], out_d.ap()[t + 1])

            if LOOP_N > 0:
                with tc.For_i(0, LOOP_N, 1):
                    body()
            else:
                body()

    nc.compile()
    return nc


def _get_nc():
    if "nc" not in _CACHE:
        _CACHE["nc"] = _build()
    return _CACHE["nc"]


def kernel(target, h0, c0, h1, c1,
           wi0, bi0, wh0, bh0,
           wi1, bi1, wh1, bh1,
           wtop, btop):
    from concourse.bass_utils import run_bass_kernel_spmd

    nc = _get_nc()

    target = np.asarray(target, np.float32)
    shared = {
        "w0": _prep_w0(np.asarray(wi0, np.float32)),
        "u0": _prep_w(np.asarray(wh0, np.float32)),
        "w1": _prep_w(np.asarray(wi1, np.float32)),
        "u1": _prep_w(np.asarray(wh1, np.float32)),
        "wt": _prep_w(np.asarray(wtop, np.float32)),
        "b0": np.ascontiguousarray(
            (np.asarray(bi0) + np.asarray(bh0)).astype(np.float32)
            .reshape(4, CH).T),
        "b1": np.ascontiguousarray(
            (np.asarray(bi1) + np.asarray(bh1)).astype(np.float32)
            .reshape(4, CH).T),
        "bt": np.asarray(btop, np.float32).reshape(C, 1),
        "zz": _host_cast(np.zeros((CH, BUFC), np.float32)),
    }
    in_maps = []
    for b in range(B):
        m = dict(shared)
        m["xs"] = _host_cast(target[b, :NSTEP].reshape(NSTEP, C, HW))
        m["h0i"] = _host_cast(np.asarray(h0, np.float32)[b].reshape(CH, HW))
        m["c0i"] = np.ascontiguousarray(
            np.asarray(c0, np.float32)[b].reshape(CH, HW))
        m["h1i"] = _host_cast(np.asarray(h1, np.float32)[b].reshape(CH, HW))
        m["c1i"] = np.ascontiguousarray(
            np.asarray(c1, np.float32)[b].reshape(CH, HW))
        in_maps.append(m)

    res = run_bass_kernel_spmd(nc, in_maps, core_ids=list(range(B)))
    out = np.stack([res.results[b]["out"].reshape(T, C, H, W)
                    for b in range(B)])
    return out



# revision 2
# speedup vs baseline: 3.6285x; 3.6285x over previous
"""ConvLSTM decoder (2 ConvLSTM layers + top conv) on 8 Trainium2 cores.

Sharding: data-parallel over batch — B=8, one batch element per core,
weights replicated. The T=10 recurrence runs fully on-core.

Layout: images are stored in SBUF as a zero-padded flat row-major strip:
each 64-pixel row padded to 66 cols (1 zero col each side), 64 rows
contiguous, plus 68-col zero margins at both ends. A 3x3 'SAME' conv then
becomes 9 shifted matmuls accumulated in PSUM: for tap (dy,dx) the rhs is
the flat strip shifted by dy*66+dx.

fp8 path (default): the four gate convs run in fp8e4 with DoubleRow
perf mode — taps are processed two at a time per matmul (K=256 packed as
2 fp8 weights/cell), with the rhs expressed as a 3-D access pattern
[K, 2, N] whose middle dim strides between the two taps' shifted strips
(HW-validated for arbitrary strides). 9 taps -> 4 DR pairs + 1 single.
The top conv stays bf16 (its output feeds rel-err directly; fp8 there
fails the 2e-2 gate, fp8 gates alone measure ~1.2e-2).

Layer-0 i2h has only 64 input channels. Its 9 taps are packed into 5
K-stacked slots (x and a shifted copy of x stacked in partitions 64:128
of two strip copies xb66/xb2, both living in one arena tile so the
DR pair (slot2,slot3) has a fixed intra-partition stride), then slots
are DR-paired: (s0,s1), (s2,s3) + lone K=64 slot s4 (normal fp8 MM).
"""

import numpy as np

B, T, C, H, W = 8, 10, 64, 64, 64
CH = 128
NSTEP = T - 1          # 9 recurrent steps
WP = W + 2             # padded row width
FLAT = H * WP          # 4224
MARG = 68              # >= 67 = max |tap offset|
BUFC = MARG + FLAT + MARG
BASE = MARG
HW = H * W             # 4096

# row chunks (r0, r1): 8x7 rows + 2x4 rows; max matmul N = 7*66 = 462 <= 512
CHUNKS = [(i * 7, i * 7 + 7) for i in range(8)] + [(56, 60), (60, 64)]
# weight-sharing groups: matmuls per ldweights = group size; <=4 banks open
CGROUPS = [(0, 4), (4, 8), (8, 10)]

TAPS = [(dy, dx) for dy in (-1, 0, 1) for dx in (-1, 0, 1)]

# std conv DR pairing: (kkA, kkB, offA, strideAB) then single center tap
#   P1..P3: (0,kx)&(2,kx) offsets -WP+dx / +WP+dx -> stride 2*WP
#   P4: (1,0)&(1,2) offsets -1/+1 -> stride 2
STD_PAIRS = [((0, 0), (2, 0), -WP - 1, 2 * WP),
             ((0, 1), (2, 1), -WP, 2 * WP),
             ((0, 2), (2, 2), -WP + 1, 2 * WP),
             ((1, 0), (1, 2), -1, 2)]
STD_SINGLE = (1, 1)    # offset 0

FP8 = True             # gates convs in fp8e4 + DoubleRow; top conv bf16
LOOP_N = 0             # >0: wrap body in a hardware repeat loop (timing only)

_CACHE = {}


def _cast_bf16(a):
    import ml_dtypes
    return np.ascontiguousarray(a.astype(ml_dtypes.bfloat16))


def _cast_fp8(a):
    import ml_dtypes
    return np.ascontiguousarray(a.astype(ml_dtypes.float8_e4m3))


def _cast_gate(a):
    return _cast_fp8(a) if FP8 else _cast_bf16(a)


def _prep_w_pairs(w):
    """[512, I, 3, 3] -> [I, 4g*4p*2*128 + 4g*128]: DR pair lhsTs then
    center-tap singles, gate-chunk-major."""
    O, I = w.shape[0], w.shape[1]
    ng = O // CH
    pairs = np.zeros((I, ng, 4, 2, CH), np.float32)
    single = np.zeros((I, ng, CH), np.float32)
    for g in range(ng):
        sl = slice(g * CH, (g + 1) * CH)
        for p, (ka, kb, _, _) in enumerate(STD_PAIRS):
            pairs[:, g, p, 0, :] = w[sl, :, ka[0], ka[1]].T
            pairs[:, g, p, 1, :] = w[sl, :, kb[0], kb[1]].T
        single[:, g, :] = w[sl, :, STD_SINGLE[0], STD_SINGLE[1]].T
    out = np.concatenate([pairs.reshape(I, -1), single.reshape(I, -1)], 1)
    return _cast_gate(out)


def _prep_w_flat(w):
    # [O, I, 3, 3] -> [I, 9*O]; slice for (tap ti, 128-chunk g): ti*O + g*128
    O, I = w.shape[0], w.shape[1]
    return _cast_bf16(w.transpose(1, 2, 3, 0).reshape(I, 9 * O))


# L0 i2h slot packing: slots s0..s3 are K=128 (x stacked with shifted x),
# s4 is the lone K=64 tap (2,1). Slot -> (ky,kx) of (low, high) partitions:
L0SLOT_KK = [((0, 0), (1, 0)), ((0, 1), (1, 1)), ((0, 2), (1, 2)),
             ((2, 0), (2, 2)), ((2, 1), None)]


def _prep_w0(w):
    """[512, 64, 3, 3] -> [128, 4g*2p*2*128 + 4g*128] slot-stacked DR pairs
    (s0,s1),(s2,s3) + K=64 single s4 (partitions 0:64)."""
    O, I = w.shape[0], w.shape[1]
    ng = O // CH
    pairs = np.zeros((2 * I, ng, 2, 2, CH), np.float32)
    single = np.zeros((2 * I, ng, CH), np.float32)
    for g in range(ng):
        sl = slice(g * CH, (g + 1) * CH)
        for p in range(2):          # pair p covers slots 2p, 2p+1
            for e in range(2):
                lo, hi = L0SLOT_KK[2 * p + e]
                pairs[:I, g, p, e, :] = w[sl, :, lo[0], lo[1]].T
                if hi is not None:
                    pairs[I:, g, p, e, :] = w[sl, :, hi[0], hi[1]].T
        lo, _ = L0SLOT_KK[4]
        single[:I, g, :] = w[sl, :, lo[0], lo[1]].T
    out = np.concatenate([pairs.reshape(2 * I, -1),
                          single.reshape(2 * I, -1)], 1)
    return _cast_gate(out)


def _build():
    import concourse.bass as bass  # noqa: F401
    import concourse.tile as tile
    from concourse import bacc, mybir
    from concourse.bass_types import AP

    f32 = mybir.dt.float32
    bf16 = mybir.dt.bfloat16
    gdt = mybir.dt.float8e4 if FP8 else bf16   # gate-conv operand dtype
    DRMODE = mybir.MatmulPerfMode.DoubleRow if FP8 else None
    AF = mybir.ActivationFunctionType

    nc = bacc.Bacc("TRN2", target_bir_lowering=False, debug=False,
                   num_devices=8)

    W0COLS = 4 * 2 * 2 * CH + 4 * CH           # 2560
    UCOLS = 4 * 4 * 2 * CH + 4 * CH            # 4608

    xs_d = nc.dram_tensor("xs", [NSTEP, C, HW], gdt, kind="ExternalInput")
    h0_d = nc.dram_tensor("h0i", [CH, HW], gdt, kind="ExternalInput")
    c0_d = nc.dram_tensor("c0i", [CH, HW], f32, kind="ExternalInput")
    h18_d = nc.dram_tensor("h1i8", [CH, HW], gdt, kind="ExternalInput")
    h16_d = nc.dram_tensor("h1i6", [CH, HW], bf16, kind="ExternalInput")
    c1_d = nc.dram_tensor("c1i", [CH, HW], f32, kind="ExternalInput")
    w0_d = nc.dram_tensor("w0", [2 * C, W0COLS], gdt, kind="ExternalInput")
    u0_d = nc.dram_tensor("u0", [CH, UCOLS], gdt, kind="ExternalInput")
    w1_d = nc.dram_tensor("w1", [CH, UCOLS], gdt, kind="ExternalInput")
    u1_d = nc.dram_tensor("u1", [CH, UCOLS], gdt, kind="ExternalInput")
    wt_d = nc.dram_tensor("wt", [CH, 9 * C], bf16, kind="ExternalInput")
    b0_d = nc.dram_tensor("b0", [CH, 4], f32, kind="ExternalInput")
    b1_d = nc.dram_tensor("b1", [CH, 4], f32, kind="ExternalInput")
    bt_d = nc.dram_tensor("bt", [C, 1], f32, kind="ExternalInput")
    out_d = nc.dram_tensor("out", [T, C, HW], f32, kind="ExternalOutput")

    def interior(ap_2d, s0, nrow):
        # rows of 64 interior cols at stride 66 starting at flat offset s0
        return ap_2d[:, s0:s0 + nrow * WP].rearrange(
            "p (r c) -> p r c", c=WP)[:, :, 1:1 + W]

    def dr_rhs(src, kk, s, stride, n):
        # [kk, 2, n] AP: pair elems at cols s and s+stride, rows contiguous
        base = src[:kk, s:s + n]
        return AP(base.tensor, base.offset,
                  [list(base.ap[0]), [stride, 2], [1, n]])

    with tile.TileContext(nc) as tc:
        with (
            tc.tile_pool(name="pers", bufs=1) as pers,
            tc.tile_pool(name="ps", bufs=8, space="PSUM") as psp,
            tc.tile_pool(name="gt", bufs=5) as gtp,
            tc.tile_pool(name="osb", bufs=5) as osbp,
        ):
            # --- persistent SBUF residents ---
            w0_t = pers.tile([2 * C, W0COLS], gdt, tag="w0")
            u0_t = pers.tile([CH, UCOLS], gdt, tag="u0")
            w1_t = pers.tile([CH, UCOLS], gdt, tag="w1")
            u1_t = pers.tile([CH, UCOLS], gdt, tag="u1")
            wt_t = pers.tile([CH, 9 * C], bf16, tag="wt")
            b0_t = pers.tile([CH, 4], f32, tag="b0")
            b1_t = pers.tile([CH, 4], f32, tag="b1")
            bt_t = pers.tile([C, 1], f32, tag="bt")
            # x arena: xb66 strip at cols [0, BUFC), xb2 at [BUFC, 2*BUFC)
            xar = pers.tile([2 * C, 2 * BUFC], gdt, tag="xar")
            h0p = [pers.tile([CH, BUFC], gdt, tag=f"h0p{i}", name=f"h0p{i}")
                   for i in range(2)]
            h1p = [pers.tile([CH, BUFC], gdt, tag=f"h1p{i}", name=f"h1p{i}")
                   for i in range(2)]
            # bf16 copies of the h1 strip for the top conv
            h1q = [pers.tile([CH, BUFC], bf16, tag=f"h1q{i}", name=f"h1q{i}")
                   for i in range(2)]
            c0_t = pers.tile([CH, HW], f32, tag="c0")
            c1_t = pers.tile([CH, HW], f32, tag="c1")

            for t_, d_ in ((w0_t, w0_d), (u0_t, u0_d), (w1_t, w1_d),
                           (u1_t, u1_d), (wt_t, wt_d), (b0_t, b0_d),
                           (b1_t, b1_d), (bt_t, bt_d)):
                nc.sync.dma_start(t_[:], d_.ap())

            # one-time zero fill (margins/padding stay zero forever; the
            # interiors are fully re-written by DMA/compute every iteration)
            for buf in (xar, h0p[0], h0p[1], h1p[0], h1p[1], h1q[0], h1q[1]):
                nc.vector.memset(buf[:], 0.0)

            def load_x(t):
                # x strip into xb66[0:64]@BASE, xb66[64:128]@BASE-66,
                # xb2[0:64]@BUFC+BASE, xb2[64:128]@BUFC+BASE-2
                src = xs_d.ap()[t]
                nc.sync.dma_start(interior(xar[:C, :], BASE, H), src)
                nc.sync.dma_start(interior(xar[C:2 * C, :], BASE - 66, H),
                                  src)
                nc.sync.dma_start(interior(xar[:C, :], BUFC + BASE, H), src)
                nc.sync.dma_start(interior(xar[C:2 * C, :], BUFC + BASE - 2,
                                           H), src)

            def init_states():
                nc.sync.dma_start(interior(h1p[0], BASE, H), h18_d.ap())
                nc.sync.dma_start(interior(h1q[0], BASE, H), h16_d.ap())
                load_x(0)
                nc.sync.dma_start(interior(h0p[0], BASE, H), h0_d.ap())
                nc.sync.dma_start(c0_t[:], c0_d.ap())
                nc.sync.dma_start(c1_t[:], c1_d.ap())

            # tap emitters: each returns a list of
            #   (lhs_ap_or_3d, src, kk, s_off, stride, perf)
            # where s_off is the rhs col offset of pair-elem0 rel. to chunk
            # base, stride None -> plain 2D rhs.
            def l0_xtaps(g):
                res = []
                for p in range(2):        # DR pairs (s0,s1), (s2,s3)
                    o = ((g * 2 + p) * 2) * CH
                    lhs = w0_t[:, o:o + 2 * CH].rearrange(
                        "p (two m) -> p two m", two=2)
                    if p == 0:
                        # slots s0/s1: xb66 @ -67 / -66 -> stride 1
                        res.append((lhs, xar, 2 * C, -67, 1, DRMODE))
                    else:
                        # s2: xb66 @ -65 ; s3: xb2 @ +65 -> stride BUFC+130
                        res.append((lhs, xar, 2 * C, -65, BUFC + 130,
                                    DRMODE))
                o = 4 * 2 * 2 * CH + g * CH
                res.append((w0_t[:C, o:o + CH], xar, C, 66, None, None))
                return res

            def std_taps(w_t, src, g, base_col=0):
                res = []
                for p, (_, _, offa, stride) in enumerate(STD_PAIRS):
                    o = ((g * 4 + p) * 2) * CH
                    lhs = w_t[:, o:o + 2 * CH].rearrange(
                        "p (two m) -> p two m", two=2)
                    res.append((lhs, src, CH, base_col + offa, stride,
                                DRMODE))
                o = 4 * 4 * 2 * CH + g * CH
                res.append((w_t[:, o:o + CH], src, CH, base_col, None, None))
                return res

            def std_taps_bf16(w_t, src, g):
                res = []
                for ti in range(9):
                    dy, dx = TAPS[ti]
                    o = ti * 4 * CH + g * CH
                    res.append((w_t[:, o:o + CH], src, CH, dy * WP + dx,
                                None, None))
                return res

            def conv_gates(xtaps_fn, hin, wh_t, b_t, c_t, houts, h_first):
                """One ConvLSTM cell; chunk groups share stationary
                weights (group-size matmuls per ldweights). houts: list of
                strip tiles the new h is written to (1 or 2)."""
                for bi, be in CGROUPS:
                    pair = CHUNKS[bi:be]
                    gtiles = [[None] * 4 for _ in pair]
                    for g in range(4):
                        pss = [psp.tile([CH, (r1 - r0) * WP], f32, tag="ps",
                                        name="ps") for (r0, r1) in pair]
                        xt = xtaps_fn(g)
                        ht = std_taps(wh_t, hin, g)
                        taps = (ht + xt) if h_first else (xt + ht)
                        nt = len(taps)
                        for k, (lhs, src, kk, off, stride, perf) in \
                                enumerate(taps):
                            for j, (r0, r1) in enumerate(pair):
                                s = BASE + r0 * WP + off
                                cw = (r1 - r0) * WP
                                if stride is None:
                                    rhs = src[:kk, s:s + cw]
                                else:
                                    rhs = dr_rhs(src, kk, s, stride, cw)
                                nc.tensor.matmul(pss[j][:], lhs, rhs,
                                                 start=(k == 0),
                                                 stop=(k == nt - 1),
                                                 perf_mode=perf)
                        for j, (r0, r1) in enumerate(pair):
                            nr = r1 - r0
                            gt = gtp.tile([CH, nr * W], f32, tag=f"g{g}",
                                          name=f"g{g}")
                            func = AF.Tanh if g == 2 else AF.Sigmoid
                            nc.scalar.activation(
                                gt[:].rearrange("p (r c) -> p r c", c=W),
                                pss[j][:].rearrange(
                                    "p (r c) -> p r c", c=WP)[:, :, 1:1 + W],
                                func, bias=b_t[:, g:g + 1])
                            gtiles[j][g] = gt
                    for j, (r0, r1) in enumerate(pair):
                        nr = r1 - r0
                        gi, gf, gg, go = gtiles[j]
                        csl = c_t[:, r0 * W:r1 * W]
                        nc.vector.tensor_mul(gg[:], gi[:], gg[:])   # i*g
                        nc.vector.tensor_mul(csl, gf[:], csl)       # f*c
                        nc.vector.tensor_add(csl, csl, gg[:])       # c
                        nc.scalar.activation(gf[:], csl, AF.Tanh)
                        for hout in houts:
                            nc.vector.tensor_mul(
                                interior(hout, BASE + r0 * WP, nr),
                                go[:].rearrange("p (r c) -> p r c", c=W),
                                gf[:].rearrange("p (r c) -> p r c", c=W))

            def conv_top(hin, tout):
                for bi, be in CGROUPS:
                    pair = CHUNKS[bi:be]
                    pss = [psp.tile([C, (r1 - r0) * WP], f32, tag="ps",
                                    name="ps") for (r0, r1) in pair]
                    for ti in range(9):
                        dy, dx = TAPS[ti]
                        lhs = wt_t[:, ti * C:(ti + 1) * C]
                        for j, (r0, r1) in enumerate(pair):
                            s = BASE + r0 * WP + dy * WP + dx
                            cw = (r1 - r0) * WP
                            nc.tensor.matmul(pss[j][:], lhs, hin[:, s:s + cw],
                                             start=(ti == 0), stop=(ti == 8))
                    for j, (r0, r1) in enumerate(pair):
                        nr = r1 - r0
                        ot = osbp.tile([C, nr * W], f32, tag="ot", name="ot")
                        nc.scalar.activation(
                            ot[:].rearrange("p (r c) -> p r c", c=W),
                            pss[j][:].rearrange(
                                "p (r c) -> p r c", c=WP)[:, :, 1:1 + W],
                            AF.Identity, bias=bt_t[:, 0:1])
                        nc.gpsimd.dma_start(tout[:, r0 * W:r1 * W], ot[:])

            def l1_xtaps_for(h0buf):
                return lambda g: std_taps(w1_t, h0buf, g)

            def body():
                init_states()
                conv_top(h1q[0], out_d.ap()[0])
                for t in range(NSTEP):
                    if t > 0:
                        load_x(t)
                    conv_gates(l0_xtaps, h0p[t % 2], u0_t, b0_t, c0_t,
                               [h0p[(t + 1) % 2]], h_first=False)
                    conv_gates(l1_xtaps_for(h0p[(t + 1) % 2]), h1p[t % 2],
                               u1_t, b1_t, c1_t,
                               [h1p[(t + 1) % 2], h1q[(t + 1) % 2]],
                               h_first=True)
                    conv_top(h1q[(t + 1) % 2], out_d.ap()[t + 1])

            if LOOP_N > 0:
                with tc.For_i(0, LOOP_N, 1):
                    body()
            else:
                body()

    nc.compile()
    return nc


def _get_nc():
    if "nc" not in _CACHE:
        _CACHE["nc"] = _build()
    return _CACHE["nc"]


def kernel(target, h0, c0, h1, c1,
           wi0, bi0, wh0, bh0,
           wi1, bi1, wh1, bh1,
           wtop, btop):
    from concourse.bass_utils import run_bass_kernel_spmd

    nc = _get_nc()

    target = np.asarray(target, np.float32)
    shared = {
        "w0": _prep_w0(np.asarray(wi0, np.float32)),
        "u0": _prep_w_pairs(np.asarray(wh0, np.float32)),
        "w1": _prep_w_pairs(np.asarray(wi1, np.float32)),
        "u1": _prep_w_pairs(np.asarray(wh1, np.float32)),
        "wt": _prep_w_flat(np.asarray(wtop, np.float32)),
        "b0": np.ascontiguousarray(
            (np.asarray(bi0) + np.asarray(bh0)).astype(np.float32)
            .reshape(4, CH).T),
        "b1": np.ascontiguousarray(
            (np.asarray(bi1) + np.asarray(bh1)).astype(np.float32)
            .reshape(4, CH).T),
        "bt": np.asarray(btop, np.float32).reshape(C, 1),
    }
    in_maps = []
    for b in range(B):
        m = dict(shared)
        m["xs"] = _cast_gate(target[b, :NSTEP].reshape(NSTEP, C, HW))
        m["h0i"] = _cast_gate(np.asarray(h0, np.float32)[b].reshape(CH, HW))
        m["c0i"] = np.ascontiguousarray(
            np.asarray(c0, np.float32)[b].reshape(CH, HW))
        h1b = np.asarray(h1, np.float32)[b].reshape(CH, HW)
        m["h1i8"] = _cast_gate(h1b)
        m["h1i6"] = _cast_bf16(h1b)
        m["c1i"] = np.ascontiguousarray(
            np.asarray(c1, np.float32)[b].reshape(CH, HW))
        in_maps.append(m)

    res = run_bass_kernel_spmd(nc, in_maps, core_ids=list(range(B)))
    out = np.stack([res.results[b]["out"].reshape(T, C, H, W)
                    for b in range(B)])
    return out


# revision 4
# speedup vs baseline: 6.5924x; 1.8168x over previous
"""ConvLSTM decoder (2 ConvLSTM layers + top conv) on 8 Trainium2 cores.

Sharding: data-parallel over batch — B=8, one batch element per core,
weights replicated. The T=10 recurrence runs fully on-core.

Layout: images live in SBUF as zero-padded flat row-major strips: each
64-pixel row padded to WP=65 cols (one shared zero col between rows),
64 rows contiguous, 68-col zero margins at both ends. A 3x3 'SAME' conv
is 9 shifted matmuls accumulated in PSUM: tap (dy,dx) reads the strip
shifted by dy*WP+dx.

All gate-conv strips (h1 x2, h0 x2, x-stacked x2) live in ONE fp8 arena
tile so any two taps — even across the i2h/h2h operands of a cell — can
form a DoubleRow fp8 pair: one matmul with K=256 (2 fp8 weights/cell),
rhs a 3-D AP [K, 2, N] whose middle dim strides between the two taps'
strip positions (HW-validated for arbitrary strides). Gate convs become
pure DR pairs: L1 = 9 MMs (4 h2h pairs + h2h/i2h center pair + 4 i2h
pairs), L0 = 7 MMs (x packed K=64->128 by partition-stacking shifted
copies; lone K=64 tap rides the h2h-center pair with upper-64 weights
zeroed). The top conv stays bf16 (fp8 there fails the 2e-2 gate; fp8
gates alone measure ~1.2e-2) and col-tiles its M=64 output: even taps
accumulate in psum partitions 0:64, odd taps in 64:128 (concurrent PE
col-groups), summed on DVE at drain.
"""

import numpy as np

B, T, C, H, W = 8, 10, 64, 64, 64
CH = 128
NSTEP = T - 1          # 9 recurrent steps
WP = W + 1             # padded row width (one shared zero col per row)
FLAT = H * WP          # 4160
MARG = 68              # >= WP+1 = max |tap offset|
BUFC = MARG + FLAT + MARG
BASE = MARG
HW = H * W             # 4096

# arena regions (fp8): [H1A, H1B, H0A, H0B, XB66, XB2], each BUFC wide
R_H1 = [0 * BUFC, 1 * BUFC]
R_H0 = [2 * BUFC, 3 * BUFC]
R_XA = 4 * BUFC        # x stacked with x shifted -WP in partitions 64:128
R_XB = 5 * BUFC        # x stacked with x shifted -2
AW = 6 * BUFC

# row chunks (r0, r1): 8x7 rows + 2x4 rows; max matmul N = 7*65 = 455 <= 512
CHUNKS = [(i * 7, i * 7 + 7) for i in range(8)] + [(56, 60), (60, 64)]
# weight-sharing groups: matmuls per ldweights = group size; <=4 banks open
CGROUPS = [(0, 4), (4, 8), (8, 10)]

TAPS = [(dy, dx) for dy in (-1, 0, 1) for dx in (-1, 0, 1)]

# std conv DR pairing: (kkA, kkB, offA, strideAB); center tap (1,1) rides
# a cross-operand pair.
STD_PAIRS = [((0, 0), (2, 0), -WP - 1, 2 * WP),
             ((0, 1), (2, 1), -WP, 2 * WP),
             ((0, 2), (2, 2), -WP + 1, 2 * WP),
             ((1, 0), (1, 2), -1, 2)]

# L0 i2h slot packing: slots s0..s3 are K=128 (x stacked with shifted x),
# s4 is the lone K=64 tap (2,1). Slot -> (ky,kx) of (low, high) partitions:
L0SLOT_KK = [((0, 0), (1, 0)), ((0, 1), (1, 1)), ((0, 2), (1, 2)),
             ((2, 0), (2, 2)), ((2, 1), None)]

TOPCT = False          # col-tile the top conv (interp psum-bank check
                       # false-positives on base-partition-64 outputs)
LOOP_N = 0             # >0: wrap body in a hardware repeat loop (timing only)

_CACHE = {}


def _cast_bf16(a):
    import ml_dtypes
    return np.ascontiguousarray(a.astype(ml_dtypes.bfloat16))


def _cast_fp8(a):
    import ml_dtypes
    return np.ascontiguousarray(a.astype(ml_dtypes.float8_e4m3))


def _prep_w_pairs(w):
    """[512, 128, 3, 3] -> [128, 4g*4p*2*128]: the 4 std DR pair lhsTs,
    gate-chunk-major (center tap handled separately)."""
    O, I = w.shape[0], w.shape[1]
    ng = O // CH
    pairs = np.zeros((I, ng, 4, 2, CH), np.float32)
    for g in range(ng):
        sl = slice(g * CH, (g + 1) * CH)
        for p, (ka, kb, _, _) in enumerate(STD_PAIRS):
            pairs[:, g, p, 0, :] = w[sl, :, ka[0], ka[1]].T
            pairs[:, g, p, 1, :] = w[sl, :, kb[0], kb[1]].T
    return _cast_fp8(pairs.reshape(I, -1))


def _prep_center_pair(wh, wx_elem1):
    """Center cross pair: elem0 = wh center tap (K=128), elem1 = wx_elem1
    [K, 4CH] (already K=128 rows, upper rows zeroed if needed).
    -> [128, 4g*2*128]."""
    out = np.zeros((CH, 4, 2, CH), np.float32)
    for g in range(4):
        sl = slice(g * CH, (g + 1) * CH)
        out[:, g, 0, :] = wh[sl, :, 1, 1].T
        out[:, g, 1, :] = wx_elem1[:, sl]
    return _cast_fp8(out.reshape(CH, -1))


def _prep_w0_pairs(w):
    """[512, 64, 3, 3] -> [128, 4g*2p*2*128] slot-stacked DR pairs
    (s0,s1),(s2,s3)."""
    O, I = w.shape[0], w.shape[1]
    ng = O // CH
    pairs = np.zeros((2 * I, ng, 2, 2, CH), np.float32)
    for g in range(ng):
        sl = slice(g * CH, (g + 1) * CH)
        for p in range(2):
            for e in range(2):
                lo, hi = L0SLOT_KK[2 * p + e]
                pairs[:I, g, p, e, :] = w[sl, :, lo[0], lo[1]].T
                if hi is not None:
                    pairs[I:, g, p, e, :] = w[sl, :, hi[0], hi[1]].T
    return _cast_fp8(pairs.reshape(2 * I, -1))


def _prep_w_flat(w):
    # [O, I, 3, 3] -> [I, 9*O]; slice for (tap ti, chunk g): ti*O + g*...
    O, I = w.shape[0], w.shape[1]
    return _cast_bf16(w.transpose(1, 2, 3, 0).reshape(I, 9 * O))


def _build():
    import concourse.bass as bass  # noqa: F401
    import concourse.tile as tile
    from concourse import bacc, mybir
    from concourse.bass_types import AP

    f32 = mybir.dt.float32
    bf16 = mybir.dt.bfloat16
    f8 = mybir.dt.float8e4
    DR = mybir.MatmulPerfMode.DoubleRow
    AF = mybir.ActivationFunctionType

    nc = bacc.Bacc("TRN2", target_bir_lowering=False, debug=False,
                   num_devices=8)

    PW = 4 * 4 * 2 * CH            # 4096: std pairs region cols
    CW = 4 * 2 * CH                # 1024: center-pair region cols
    P0W = 4 * 2 * 2 * CH           # 2048: L0 i2h pairs cols

    xs_d = nc.dram_tensor("xs", [NSTEP, C, HW], f8, kind="ExternalInput")
    h0_d = nc.dram_tensor("h0i", [CH, HW], f8, kind="ExternalInput")
    c0_d = nc.dram_tensor("c0i", [CH, HW], f32, kind="ExternalInput")
    h18_d = nc.dram_tensor("h1i8", [CH, HW], f8, kind="ExternalInput")
    h16_d = nc.dram_tensor("h1i6", [CH, HW], bf16, kind="ExternalInput")
    c1_d = nc.dram_tensor("c1i", [CH, HW], f32, kind="ExternalInput")
    w0_d = nc.dram_tensor("w0", [2 * C, P0W], f8, kind="ExternalInput")
    u0_d = nc.dram_tensor("u0", [CH, PW + CW], f8, kind="ExternalInput")
    w1_d = nc.dram_tensor("w1", [CH, PW], f8, kind="ExternalInput")
    u1_d = nc.dram_tensor("u1", [CH, PW + CW], f8, kind="ExternalInput")
    wt_d = nc.dram_tensor("wt", [CH, 9 * C], bf16, kind="ExternalInput")
    b0_d = nc.dram_tensor("b0", [CH, 4], f32, kind="ExternalInput")
    b1_d = nc.dram_tensor("b1", [CH, 4], f32, kind="ExternalInput")
    bt_d = nc.dram_tensor("bt", [C, 1], f32, kind="ExternalInput")
    out_d = nc.dram_tensor("out", [T, C, HW], f32, kind="ExternalOutput")

    def interior(ap_2d, s0, nrow):
        # rows of 64 interior cols at stride WP starting at flat offset s0
        return ap_2d[:, s0:s0 + nrow * WP].rearrange(
            "p (r c) -> p r c", c=WP)[:, :, 1:1 + W]

    with tile.TileContext(nc) as tc:
        with (
            tc.tile_pool(name="pers", bufs=1) as pers,
            tc.tile_pool(name="ps", bufs=8, space="PSUM") as psp,
            tc.tile_pool(name="gt", bufs=5) as gtp,
            tc.tile_pool(name="osb", bufs=6) as osbp,
        ):
            # --- persistent SBUF residents ---
            w0_t = pers.tile([2 * C, P0W], f8, tag="w0")
            u0_t = pers.tile([CH, PW + CW], f8, tag="u0")
            w1_t = pers.tile([CH, PW], f8, tag="w1")
            u1_t = pers.tile([CH, PW + CW], f8, tag="u1")
            wt_t = pers.tile([CH, 9 * C], bf16, tag="wt")
            b0_t = pers.tile([CH, 4], f32, tag="b0")
            b1_t = pers.tile([CH, 4], f32, tag="b1")
            bt_t = pers.tile([C, 1], f32, tag="bt")
            ar = pers.tile([CH, AW], f8, tag="ar")
            # bf16 copies of the h1 strip for the top conv
            h1q = [pers.tile([CH, BUFC], bf16, tag=f"h1q{i}", name=f"h1q{i}")
                   for i in range(2)]
            c0_t = pers.tile([CH, HW], f32, tag="c0")
            c1_t = pers.tile([CH, HW], f32, tag="c1")

            for t_, d_ in ((w0_t, w0_d), (u0_t, u0_d), (w1_t, w1_d),
                           (u1_t, u1_d), (wt_t, wt_d), (b0_t, b0_d),
                           (b1_t, b1_d), (bt_t, bt_d)):
                nc.sync.dma_start(t_[:], d_.ap())

            # one-time zero fill (margins/padding stay zero forever; the
            # interiors are fully re-written by DMA/compute every iteration)
            nc.vector.memset(ar[:], 0.0)
            for buf in h1q:
                nc.vector.memset(buf[:], 0.0)

            def load_x(t):
                src = xs_d.ap()[t]
                nc.sync.dma_start(interior(ar[:C, :], R_XA + BASE, H), src)
                nc.sync.dma_start(
                    interior(ar[C:2 * C, :], R_XA + BASE - WP, H), src)
                nc.sync.dma_start(interior(ar[:C, :], R_XB + BASE, H), src)
                nc.sync.dma_start(
                    interior(ar[C:2 * C, :], R_XB + BASE - 2, H), src)

            def init_states():
                nc.sync.dma_start(interior(ar, R_H1[0] + BASE, H),
                                  h18_d.ap())
                nc.sync.dma_start(interior(h1q[0], BASE, H), h16_d.ap())
                load_x(0)
                nc.sync.dma_start(interior(ar, R_H0[0] + BASE, H),
                                  h0_d.ap())
                nc.sync.dma_start(c0_t[:], c0_d.ap())
                nc.sync.dma_start(c1_t[:], c1_d.ap())

            def dr_rhs(kk, s, stride, n):
                base = ar[:kk, s:s + n]
                return AP(base.tensor, base.offset,
                          [list(base.ap[0]), [stride, 2], [1, n]])

            def pair3(w_t, o):
                return w_t[:, o:o + 2 * CH].rearrange(
                    "p (two m) -> p two m", two=2)

            # tap emitters: lists of (lhs3d, kk, abs_off_elem0, stride)
            def l0_taps(g, h0base):
                res = []
                # i2h pairs: (s0@XA-WP-1, s1@XA-WP) stride 1;
                #            (s2@XA-WP+1, s3@XB+WP-1) stride BUFC+2WP-2
                res.append((pair3(w0_t, (g * 2) * 2 * CH), 2 * C,
                            R_XA - WP - 1, 1))
                res.append((pair3(w0_t, (g * 2 + 1) * 2 * CH), 2 * C,
                            R_XA - WP + 1, R_XB - R_XA + 2 * WP - 2))
                # center-combined: h2h center @ h0base, x s4 @ XA+WP
                res.append((pair3(u0_t, PW + g * 2 * CH), CH,
                            h0base, R_XA + WP - h0base))
                # h2h std pairs on h0base strip
                for p, (_, _, offa, st) in enumerate(STD_PAIRS):
                    res.append((pair3(u0_t, (g * 4 + p) * 2 * CH), CH,
                                h0base + offa, st))
                return res

            def l1_taps(g, h1base, h0base):
                res = []
                # h2h std pairs on h1 strip
                for p, (_, _, offa, st) in enumerate(STD_PAIRS):
                    res.append((pair3(u1_t, (g * 4 + p) * 2 * CH), CH,
                                h1base + offa, st))
                # center pair: h2h center @ h1base, i2h center @ h0base
                res.append((pair3(u1_t, PW + g * 2 * CH), CH,
                            h1base, h0base - h1base))
                # i2h std pairs on h0 (x1) strip
                for p, (_, _, offa, st) in enumerate(STD_PAIRS):
                    res.append((pair3(w1_t, (g * 4 + p) * 2 * CH), CH,
                                h0base + offa, st))
                return res

            def conv_gates(taps_fn, b_t, c_t, houts):
                """One ConvLSTM cell; chunk groups share stationary
                weights. houts: [(tile_or_arena, base_col), ...]."""
                for bi, be in CGROUPS:
                    pair = CHUNKS[bi:be]
                    gtiles = [[None] * 4 for _ in pair]
                    for g in range(4):
                        pss = [psp.tile([CH, (r1 - r0) * WP], f32, tag="ps",
                                        name="ps") for (r0, r1) in pair]
                        taps = taps_fn(g)
                        nt = len(taps)
                        for k, (lhs, kk, off, stride) in enumerate(taps):
                            for j, (r0, r1) in enumerate(pair):
                                s = BASE + r0 * WP + off
                                cw = (r1 - r0) * WP
                                nc.tensor.matmul(
                                    pss[j][:], lhs,
                                    dr_rhs(kk, s, stride, cw),
                                    start=(k == 0), stop=(k == nt - 1),
                                    perf_mode=DR)
                        for j, (r0, r1) in enumerate(pair):
                            nr = r1 - r0
                            gt = gtp.tile([CH, nr * W], f32, tag=f"g{g}",
                                          name=f"g{g}")
                            func = AF.Tanh if g == 2 else AF.Sigmoid
                            nc.scalar.activation(
                                gt[:].rearrange("p (r c) -> p r c", c=W),
                                pss[j][:].rearrange(
                                    "p (r c) -> p r c", c=WP)[:, :, 1:1 + W],
                                func, bias=b_t[:, g:g + 1])
                            gtiles[j][g] = gt
                    for j, (r0, r1) in enumerate(pair):
                        nr = r1 - r0
                        gi, gf, gg, go = gtiles[j]
                        csl = c_t[:, r0 * W:r1 * W]
                        nc.vector.tensor_mul(gg[:], gi[:], gg[:])   # i*g
                        nc.vector.tensor_mul(csl, gf[:], csl)       # f*c
                        nc.vector.tensor_add(csl, csl, gg[:])       # c
                        nc.scalar.activation(gf[:], csl, AF.Tanh)
                        for htile, hb in houts:
                            nc.vector.tensor_mul(
                                interior(htile, hb + BASE + r0 * WP, nr),
                                go[:].rearrange("p (r c) -> p r c", c=W),
                                gf[:].rearrange("p (r c) -> p r c", c=W))

            def conv_top(hin, tout):
                for bi, be in CGROUPS:
                    pair = CHUNKS[bi:be]
                    if TOPCT:
                        pss = [psp.tile([2 * C, (r1 - r0) * WP], f32,
                                        tag="ps", name="ps")
                               for (r0, r1) in pair]
                        for ti in range(9):
                            dy, dx = TAPS[ti]
                            lhs = wt_t[:, ti * C:(ti + 1) * C]
                            half = ti % 2
                            for j, (r0, r1) in enumerate(pair):
                                s = BASE + r0 * WP + dy * WP + dx
                                cw = (r1 - r0) * WP
                                nc.tensor.matmul(
                                    pss[j][half * C:half * C + C, :],
                                    lhs, hin[:, s:s + cw],
                                    start=(ti < 2), stop=(ti >= 7),
                                    skip_group_check=True)
                        for j, (r0, r1) in enumerate(pair):
                            nr = r1 - r0
                            ot = osbp.tile([C, nr * W], f32, tag="ot",
                                           name="ot")
                            pv = pss[j][:].rearrange(
                                "p (r c) -> p r c", c=WP)[:, :, 1:1 + W]
                            nc.vector.tensor_add(
                                ot[:].rearrange("p (r c) -> p r c", c=W),
                                pv[0:C], pv[C:2 * C])
                            o2 = osbp.tile([C, nr * W], f32, tag="o2",
                                           name="o2")
                            nc.scalar.activation(o2[:], ot[:], AF.Identity,
                                                 bias=bt_t[:, 0:1])
                            nc.gpsimd.dma_start(tout[:, r0 * W:r1 * W],
                                                o2[:])
                    else:
                        pss = [psp.tile([C, (r1 - r0) * WP], f32, tag="ps",
                                        name="ps") for (r0, r1) in pair]
                        for ti in range(9):
                            dy, dx = TAPS[ti]
                            lhs = wt_t[:, ti * C:(ti + 1) * C]
                            for j, (r0, r1) in enumerate(pair):
                                s = BASE + r0 * WP + dy * WP + dx
                                cw = (r1 - r0) * WP
                                nc.tensor.matmul(pss[j][:], lhs,
                                                 hin[:, s:s + cw],
                                                 start=(ti == 0),
                                                 stop=(ti == 8))
                        for j, (r0, r1) in enumerate(pair):
                            nr = r1 - r0
                            ot = osbp.tile([C, nr * W], f32, tag="ot",
                                           name="ot")
                            nc.scalar.activation(
                                ot[:].rearrange("p (r c) -> p r c", c=W),
                                pss[j][:].rearrange(
                                    "p (r c) -> p r c", c=WP)[:, :, 1:1 + W],
                                AF.Identity, bias=bt_t[:, 0:1])
                            nc.gpsimd.dma_start(tout[:, r0 * W:r1 * W],
                                                ot[:])

            def body():
                init_states()
                conv_top(h1q[0][:], out_d.ap()[0])
                for t in range(NSTEP):
                    if t > 0:
                        load_x(t)
                    a, b = t % 2, (t + 1) % 2
                    conv_gates(lambda g: l0_taps(g, R_H0[a]),
                               b0_t, c0_t, [(ar, R_H0[b])])
                    conv_gates(lambda g: l1_taps(g, R_H1[a], R_H0[b]),
                               b1_t, c1_t,
                               [(ar, R_H1[b]), (h1q[b], 0)])
                    conv_top(h1q[b][:], out_d.ap()[t + 1])

            if LOOP_N > 0:
                with tc.For_i(0, LOOP_N, 1):
                    body()
            else:
                body()

    nc.compile()
    return nc


def _get_nc():
    if "nc" not in _CACHE:
        _CACHE["nc"] = _build()
    return _CACHE["nc"]


def kernel(target, h0, c0, h1, c1,
           wi0, bi0, wh0, bh0,
           wi1, bi1, wh1, bh1,
           wtop, btop):
    from concourse.bass_utils import run_bass_kernel_spmd

    nc = _get_nc()

    target = np.asarray(target, np.float32)
    wi0 = np.asarray(wi0, np.float32)
    wh0 = np.asarray(wh0, np.float32)
    wi1 = np.asarray(wi1, np.float32)
    wh1 = np.asarray(wh1, np.float32)

    # L0 center-pair elem1: lone x tap (2,1), K=64 padded to 128 with zeros
    s4 = np.zeros((CH, 4 * CH), np.float32)
    s4[:C, :] = wi0[:, :, 2, 1].T
    # L1 center-pair elem1: i2h center tap (K=128)
    w1c = wi1[:, :, 1, 1].T

    shared = {
        "w0": _prep_w0_pairs(wi0),
        "u0": np.concatenate([_prep_w_pairs(wh0),
                              _prep_center_pair(wh0, s4)], axis=1),
        "w1": _prep_w_pairs(wi1),
        "u1": np.concatenate([_prep_w_pairs(wh1),
                              _prep_center_pair(wh1, w1c)], axis=1),
        "wt": _prep_w_flat(np.asarray(wtop, np.float32)),
        "b0": np.ascontiguousarray(
            (np.asarray(bi0) + np.asarray(bh0)).astype(np.float32)
            .reshape(4, CH).T),
        "b1": np.ascontiguousarray(
            (np.asarray(bi1) + np.asarray(bh1)).astype(np.float32)
            .reshape(4, CH).T),
        "bt": np.asarray(btop, np.float32).reshape(C, 1),
    }
    in_maps = []
    for b in range(B):
        m = dict(shared)
        m["xs"] = _cast_fp8(target[b, :NSTEP].reshape(NSTEP, C, HW))
        m["h0i"] = _cast_fp8(np.asarray(h0, np.float32)[b].reshape(CH, HW))
        m["c0i"] = np.ascontiguousarray(
            np.asarray(c0, np.float32)[b].reshape(CH, HW))
        h1b = np.asarray(h1, np.float32)[b].reshape(CH, HW)
        m["h1i8"] = _cast_fp8(h1b)
        m["h1i6"] = _cast_bf16(h1b)
        m["c1i"] = np.ascontiguousarray(
            np.asarray(c1, np.float32)[b].reshape(CH, HW))
        in_maps.append(m)

    res = run_bass_kernel_spmd(nc, in_maps, core_ids=list(range(B)))
    out = np.stack([res.results[b]["out"].reshape(T, C, H, W)
                    for b in range(B)])
    return out


# revision 7
# speedup vs baseline: 8.3222x; 1.2624x over previous
"""ConvLSTM decoder (2 ConvLSTM layers + top conv) on 8 Trainium2 cores.

Sharding: data-parallel over batch — B=8, one batch element per core,
weights replicated. The T=10 recurrence runs fully on-core.

Layout: images live in SBUF as zero-padded flat row-major strips: each
64-pixel row padded to WP=65 cols (one shared zero col between rows),
64 rows contiguous, 68-col zero margins at both ends. A 3x3 'SAME' conv
is 9 shifted matmuls accumulated in PSUM: tap (dy,dx) reads the strip
shifted by dy*WP+dx.

All gate-conv strips (h1 x2, h0 x2, x-stacked x2) live in ONE fp8 arena
tile so any two taps — even across the i2h/h2h operands of a cell — can
form a DoubleRow fp8 pair: one matmul with K=256 (2 fp8 weights/cell),
rhs a 3-D AP [K, 2, N] whose middle dim strides between the two taps'
strip positions (HW-validated for arbitrary strides). Gate convs become
pure DR pairs: L1 = 9 MMs (4 h2h pairs + h2h/i2h center pair + 4 i2h
pairs), L0 = 7 MMs (x packed K=64->128 by partition-stacking shifted
copies; lone K=64 tap rides the h2h-center pair with upper-64 weights
zeroed). The top conv stays bf16 (fp8 there fails the 2e-2 gate; fp8
gates alone measure ~1.2e-2) and col-tiles its M=64 output: even taps
accumulate in psum partitions 0:64, odd taps in 64:128 (concurrent PE
col-groups), summed on DVE at drain.
"""

import numpy as np

B, T, C, H, W = 8, 10, 64, 64, 64
CH = 128
NSTEP = T - 1          # 9 recurrent steps
WP = W + 1             # padded row width (one shared zero col per row)
FLAT = H * WP          # 4160
MARG = 68              # >= WP+1 = max |tap offset|
BUFC = MARG + FLAT + MARG
BASE = MARG
HW = H * W             # 4096

# arena regions (fp8): [H1A, H1B, H0A, H0B, XB66, XB2], each BUFC wide
R_H1 = [0 * BUFC, 1 * BUFC]
R_H0 = [2 * BUFC, 3 * BUFC]
R_XA = 4 * BUFC        # x stacked with x shifted -WP in partitions 64:128
R_XB = 5 * BUFC        # x stacked with x shifted -2
AW = 6 * BUFC

# row chunks (r0, r1): 8x7 rows + 2x4 rows; max matmul N = 7*65 = 455 <= 512
CHUNKS = [(i * 7, i * 7 + 7) for i in range(8)] + [(56, 60), (60, 64)]
# weight-sharing groups: matmuls per ldweights = group size; <=4 banks open
CGROUPS = [(0, 4), (4, 8), (8, 10)]

TAPS = [(dy, dx) for dy in (-1, 0, 1) for dx in (-1, 0, 1)]

# std conv DR pairing: (kkA, kkB, offA, strideAB); center tap (1,1) rides
# a cross-operand pair.
STD_PAIRS = [((0, 0), (2, 0), -WP - 1, 2 * WP),
             ((0, 1), (2, 1), -WP, 2 * WP),
             ((0, 2), (2, 2), -WP + 1, 2 * WP),
             ((1, 0), (1, 2), -1, 2)]

# L0 i2h slot packing: slots s0..s3 are K=128 (x stacked with shifted x),
# s4 is the lone K=64 tap (2,1). Slot -> (ky,kx) of (low, high) partitions:
L0SLOT_KK = [((0, 0), (1, 0)), ((0, 1), (1, 1)), ((0, 2), (1, 2)),
             ((2, 0), (2, 2)), ((2, 1), None)]

TOPCT = True          # col-tile the top conv (interp psum-bank check
                       # false-positives on base-partition-64 outputs)
LOOP_N = 0             # >0: wrap body in a hardware repeat loop (timing only)

_CACHE = {}


def _cast_bf16(a):
    import ml_dtypes
    return np.ascontiguousarray(a.astype(ml_dtypes.bfloat16))


def _cast_fp8(a):
    import ml_dtypes
    return np.ascontiguousarray(a.astype(ml_dtypes.float8_e4m3))


def _prep_w_pairs(w):
    """[512, 128, 3, 3] -> [128, 4g*4p*2*128]: the 4 std DR pair lhsTs,
    gate-chunk-major (center tap handled separately)."""
    O, I = w.shape[0], w.shape[1]
    ng = O // CH
    pairs = np.zeros((I, ng, 4, 2, CH), np.float32)
    for g in range(ng):
        sl = slice(g * CH, (g + 1) * CH)
        for p, (ka, kb, _, _) in enumerate(STD_PAIRS):
            pairs[:, g, p, 0, :] = w[sl, :, ka[0], ka[1]].T
            pairs[:, g, p, 1, :] = w[sl, :, kb[0], kb[1]].T
    return _cast_fp8(pairs.reshape(I, -1))


def _prep_center_pair(wh, wx_elem1):
    """Center cross pair: elem0 = wh center tap (K=128), elem1 = wx_elem1
    [K, 4CH] (already K=128 rows, upper rows zeroed if needed).
    -> [128, 4g*2*128]."""
    out = np.zeros((CH, 4, 2, CH), np.float32)
    for g in range(4):
        sl = slice(g * CH, (g + 1) * CH)
        out[:, g, 0, :] = wh[sl, :, 1, 1].T
        out[:, g, 1, :] = wx_elem1[:, sl]
    return _cast_fp8(out.reshape(CH, -1))


def _prep_w0_pairs(w):
    """[512, 64, 3, 3] -> [128, 4g*2p*2*128] slot-stacked DR pairs
    (s0,s1),(s2,s3)."""
    O, I = w.shape[0], w.shape[1]
    ng = O // CH
    pairs = np.zeros((2 * I, ng, 2, 2, CH), np.float32)
    for g in range(ng):
        sl = slice(g * CH, (g + 1) * CH)
        for p in range(2):
            for e in range(2):
                lo, hi = L0SLOT_KK[2 * p + e]
                pairs[:I, g, p, e, :] = w[sl, :, lo[0], lo[1]].T
                if hi is not None:
                    pairs[I:, g, p, e, :] = w[sl, :, hi[0], hi[1]].T
    return _cast_fp8(pairs.reshape(2 * I, -1))


def _prep_w_flat(w):
    # [O, I, 3, 3] -> [I, 9*O]; slice for (tap ti, chunk g): ti*O + g*...
    O, I = w.shape[0], w.shape[1]
    return _cast_bf16(w.transpose(1, 2, 3, 0).reshape(I, 9 * O))


def _build():
    import concourse.bass as bass  # noqa: F401
    import concourse.tile as tile
    from concourse import bacc, mybir
    from concourse.bass_types import AP

    f32 = mybir.dt.float32
    bf16 = mybir.dt.bfloat16
    f8 = mybir.dt.float8e4
    DR = mybir.MatmulPerfMode.DoubleRow
    AF = mybir.ActivationFunctionType

    nc = bacc.Bacc("TRN2", target_bir_lowering=False, debug=False,
                   num_devices=8)

    PW = 4 * 4 * 2 * CH            # 4096: std pairs region cols
    CW = 4 * 2 * CH                # 1024: center-pair region cols
    P0W = 4 * 2 * 2 * CH           # 2048: L0 i2h pairs cols

    xs_d = nc.dram_tensor("xs", [NSTEP, C, HW], f8, kind="ExternalInput")
    h0_d = nc.dram_tensor("h0i", [CH, HW], f8, kind="ExternalInput")
    c0_d = nc.dram_tensor("c0i", [CH, HW], f32, kind="ExternalInput")
    h18_d = nc.dram_tensor("h1i8", [CH, HW], f8, kind="ExternalInput")
    h16_d = nc.dram_tensor("h1i6", [CH, HW], bf16, kind="ExternalInput")
    c1_d = nc.dram_tensor("c1i", [CH, HW], f32, kind="ExternalInput")
    w0_d = nc.dram_tensor("w0", [2 * C, P0W], f8, kind="ExternalInput")
    u0_d = nc.dram_tensor("u0", [CH, PW + CW], f8, kind="ExternalInput")
    w1_d = nc.dram_tensor("w1", [CH, PW], f8, kind="ExternalInput")
    u1_d = nc.dram_tensor("u1", [CH, PW + CW], f8, kind="ExternalInput")
    wt_d = nc.dram_tensor("wt", [CH, 9 * C], bf16, kind="ExternalInput")
    b0_d = nc.dram_tensor("b0", [CH, 4], f32, kind="ExternalInput")
    b1_d = nc.dram_tensor("b1", [CH, 4], f32, kind="ExternalInput")
    bt_d = nc.dram_tensor("bt", [2 * C, 1], f32, kind="ExternalInput")
    out_d = nc.dram_tensor("out", [T, C, HW], f32, kind="ExternalOutput")

    def interior(ap_2d, s0, nrow):
        # rows of 64 interior cols at stride WP starting at flat offset s0
        return ap_2d[:, s0:s0 + nrow * WP].rearrange(
            "p (r c) -> p r c", c=WP)[:, :, 1:1 + W]

    with tile.TileContext(nc) as tc:
        with (
            tc.tile_pool(name="pers", bufs=1) as pers,
            tc.tile_pool(name="ps", bufs=8, space="PSUM") as psp,
            tc.tile_pool(name="gt", bufs=5) as gtp,
            tc.tile_pool(name="osb", bufs=6) as osbp,
        ):
            # --- persistent SBUF residents ---
            w0_t = pers.tile([2 * C, P0W], f8, tag="w0")
            u0_t = pers.tile([CH, PW + CW], f8, tag="u0")
            w1_t = pers.tile([CH, PW], f8, tag="w1")
            u1_t = pers.tile([CH, PW + CW], f8, tag="u1")
            wt_t = pers.tile([CH, 9 * C], bf16, tag="wt")
            b0_t = pers.tile([CH, 4], f32, tag="b0")
            b1_t = pers.tile([CH, 4], f32, tag="b1")
            bt_t = pers.tile([2 * C, 1], f32, tag="bt")
            ar = pers.tile([CH, AW], f8, tag="ar")
            # bf16 copies of the h1 strip for the top conv
            h1q = [pers.tile([CH, BUFC], bf16, tag=f"h1q{i}", name=f"h1q{i}")
                   for i in range(2)]
            c0_t = pers.tile([CH, HW], f32, tag="c0")
            c1_t = pers.tile([CH, HW], f32, tag="c1")

            for t_, d_ in ((w0_t, w0_d), (u0_t, u0_d), (w1_t, w1_d),
                           (u1_t, u1_d), (wt_t, wt_d), (b0_t, b0_d),
                           (b1_t, b1_d), (bt_t, bt_d)):
                nc.sync.dma_start(t_[:], d_.ap())

            # one-time zero fill (margins/padding stay zero forever; the
            # interiors are fully re-written by DMA/compute every iteration)
            nc.vector.memset(ar[:], 0.0)
            for buf in h1q:
                nc.vector.memset(buf[:], 0.0)

            def load_x(t):
                src = xs_d.ap()[t]
                nc.sync.dma_start(interior(ar[:C, :], R_XA + BASE, H), src)
                nc.sync.dma_start(
                    interior(ar[C:2 * C, :], R_XA + BASE - WP, H), src)
                nc.sync.dma_start(interior(ar[:C, :], R_XB + BASE, H), src)
                nc.sync.dma_start(
                    interior(ar[C:2 * C, :], R_XB + BASE - 2, H), src)

            def init_states():
                nc.sync.dma_start(interior(ar, R_H1[0] + BASE, H),
                                  h18_d.ap())
                nc.sync.dma_start(interior(h1q[0], BASE, H), h16_d.ap())
                load_x(0)
                nc.sync.dma_start(interior(ar, R_H0[0] + BASE, H),
                                  h0_d.ap())
                nc.sync.dma_start(c0_t[:], c0_d.ap())
                nc.sync.dma_start(c1_t[:], c1_d.ap())

            def dr_rhs(kk, s, stride, n):
                base = ar[:kk, s:s + n]
                return AP(base.tensor, base.offset,
                          [list(base.ap[0]), [stride, 2], [1, n]])

            def pair3(w_t, o):
                return w_t[:, o:o + 2 * CH].rearrange(
                    "p (two m) -> p two m", two=2)

            # tap emitters: lists of (lhs3d, kk, abs_off_elem0, stride)
            def l0_taps(g, h0base):
                res = []
                # i2h pairs: (s0@XA-WP-1, s1@XA-WP) stride 1;
                #            (s2@XA-WP+1, s3@XB+WP-1) stride BUFC+2WP-2
                res.append((pair3(w0_t, (g * 2) * 2 * CH), 2 * C,
                            R_XA - WP - 1, 1))
                res.append((pair3(w0_t, (g * 2 + 1) * 2 * CH), 2 * C,
                            R_XA - WP + 1, R_XB - R_XA + 2 * WP - 2))
                # center-combined: h2h center @ h0base, x s4 @ XA+WP
                res.append((pair3(u0_t, PW + g * 2 * CH), CH,
                            h0base, R_XA + WP - h0base))
                # h2h std pairs on h0base strip
                for p, (_, _, offa, st) in enumerate(STD_PAIRS):
                    res.append((pair3(u0_t, (g * 4 + p) * 2 * CH), CH,
                                h0base + offa, st))
                return res

            def l1_taps(g, h1base, h0base):
                res = []
                # h2h std pairs on h1 strip
                for p, (_, _, offa, st) in enumerate(STD_PAIRS):
                    res.append((pair3(u1_t, (g * 4 + p) * 2 * CH), CH,
                                h1base + offa, st))
                # center pair: h2h center @ h1base, i2h center @ h0base
                res.append((pair3(u1_t, PW + g * 2 * CH), CH,
                            h1base, h0base - h1base))
                # i2h std pairs on h0 (x1) strip
                for p, (_, _, offa, st) in enumerate(STD_PAIRS):
                    res.append((pair3(w1_t, (g * 4 + p) * 2 * CH), CH,
                                h0base + offa, st))
                return res

            def conv_gates(taps_fn, b_t, c_t, houts):
                """One ConvLSTM cell; chunk groups share stationary
                weights. houts: [(tile_or_arena, base_col), ...]."""
                for bi, be in CGROUPS:
                    pair = CHUNKS[bi:be]
                    gtiles = [[None] * 4 for _ in pair]
                    for g in range(4):
                        pss = [psp.tile([CH, (r1 - r0) * WP], f32, tag="ps",
                                        name="ps") for (r0, r1) in pair]
                        taps = taps_fn(g)
                        nt = len(taps)
                        for k, (lhs, kk, off, stride) in enumerate(taps):
                            for j, (r0, r1) in enumerate(pair):
                                s = BASE + r0 * WP + off
                                cw = (r1 - r0) * WP
                                nc.tensor.matmul(
                                    pss[j][:], lhs,
                                    dr_rhs(kk, s, stride, cw),
                                    start=(k == 0), stop=(k == nt - 1),
                                    perf_mode=DR)
                        for j, (r0, r1) in enumerate(pair):
                            nr = r1 - r0
                            gt = gtp.tile([CH, nr * W], f32, tag=f"g{g}",
                                          name=f"g{g}")
                            func = AF.Tanh if g == 2 else AF.Sigmoid
                            nc.scalar.activation(
                                gt[:].rearrange("p (r c) -> p r c", c=W),
                                pss[j][:].rearrange(
                                    "p (r c) -> p r c", c=WP)[:, :, 1:1 + W],
                                func, bias=b_t[:, g:g + 1])
                            gtiles[j][g] = gt
                    for j, (r0, r1) in enumerate(pair):
                        nr = r1 - r0
                        gi, gf, gg, go = gtiles[j]
                        csl = c_t[:, r0 * W:r1 * W]
                        nc.vector.tensor_mul(gg[:], gi[:], gg[:])   # i*g
                        nc.vector.tensor_mul(csl, gf[:], csl)       # f*c
                        nc.vector.tensor_add(csl, csl, gg[:])       # c
                        nc.scalar.activation(gf[:], csl, AF.Tanh)
                        for htile, hb in houts:
                            nc.vector.tensor_mul(
                                interior(htile, hb + BASE + r0 * WP, nr),
                                go[:].rearrange("p (r c) -> p r c", c=W),
                                gf[:].rearrange("p (r c) -> p r c", c=W))

            def conv_top_ct(hin, tout):
                # col-tiled: chunk 2i in PE col-group 0 (psum parts 0:64),
                # chunk 2i+1 in col-group 1 (parts 64:128); same weights per
                # tap, both chains run concurrently on disjoint array halves.
                for pi in range(5):
                    ra, rb = CHUNKS[2 * pi], CHUNKS[2 * pi + 1]
                    nr = ra[1] - ra[0]
                    cw = nr * WP
                    ps = psp.tile([2 * C, cw], f32, tag="ps", name="ps")
                    for ti in range(9):
                        dy, dx = TAPS[ti]
                        lhs = wt_t[:, ti * C:(ti + 1) * C]
                        for half, (r0, _) in ((0, ra), (1, rb)):
                            s = BASE + r0 * WP + dy * WP + dx
                            nc.tensor.matmul(
                                ps[half * C:half * C + C, :],
                                lhs, hin[:, s:s + cw],
                                start=(ti == 0), stop=(ti == 8),
                                skip_group_check=True)
                    ot = osbp.tile([2 * C, nr * W], f32, tag="ot",
                                   name="ot")
                    nc.scalar.activation(
                        ot[:].rearrange("p (r c) -> p r c", c=W),
                        ps[:].rearrange(
                            "p (r c) -> p r c", c=WP)[:, :, 1:1 + W],
                        AF.Identity, bias=bt_t[:, 0:1])
                    nc.gpsimd.dma_start(tout[:, ra[0] * W:ra[1] * W],
                                        ot[:C])
                    nc.gpsimd.dma_start(tout[:, rb[0] * W:rb[1] * W],
                                        ot[C:2 * C])

            def conv_top(hin, tout):
                if TOPCT:
                    conv_top_ct(hin, tout)
                    return
                for bi, be in CGROUPS:
                    pair = CHUNKS[bi:be]
                    pss = [psp.tile([C, (r1 - r0) * WP], f32, tag="ps",
                                    name="ps") for (r0, r1) in pair]
                    for ti in range(9):
                        dy, dx = TAPS[ti]
                        lhs = wt_t[:, ti * C:(ti + 1) * C]
                        for j, (r0, r1) in enumerate(pair):
                            s = BASE + r0 * WP + dy * WP + dx
                            cw = (r1 - r0) * WP
                            nc.tensor.matmul(pss[j][:], lhs,
                                             hin[:, s:s + cw],
                                             start=(ti == 0),
                                             stop=(ti == 8))
                    for j, (r0, r1) in enumerate(pair):
                        nr = r1 - r0
                        ot = osbp.tile([C, nr * W], f32, tag="ot",
                                       name="ot")
                        nc.scalar.activation(
                            ot[:].rearrange("p (r c) -> p r c", c=W),
                            pss[j][:].rearrange(
                                "p (r c) -> p r c", c=WP)[:, :, 1:1 + W],
                            AF.Identity, bias=bt_t[:C, 0:1])
                        nc.gpsimd.dma_start(tout[:, r0 * W:r1 * W],
                                            ot[:])

            def body():
                init_states()
                conv_top(h1q[0][:], out_d.ap()[0])
                for t in range(NSTEP):
                    if t > 0:
                        load_x(t)
                    a, b = t % 2, (t + 1) % 2
                    conv_gates(lambda g: l0_taps(g, R_H0[a]),
                               b0_t, c0_t, [(ar, R_H0[b])])
                    conv_gates(lambda g: l1_taps(g, R_H1[a], R_H0[b]),
                               b1_t, c1_t,
                               [(ar, R_H1[b]), (h1q[b], 0)])
                    conv_top(h1q[b][:], out_d.ap()[t + 1])

            if LOOP_N > 0:
                with tc.For_i(0, LOOP_N, 1):
                    body()
            else:
                body()

    nc.compile()
    return nc


def _get_nc():
    if "nc" not in _CACHE:
        _CACHE["nc"] = _build()
    return _CACHE["nc"]


def kernel(target, h0, c0, h1, c1,
           wi0, bi0, wh0, bh0,
           wi1, bi1, wh1, bh1,
           wtop, btop):
    from concourse.bass_utils import run_bass_kernel_spmd

    nc = _get_nc()

    target = np.asarray(target, np.float32)
    wi0 = np.asarray(wi0, np.float32)
    wh0 = np.asarray(wh0, np.float32)
    wi1 = np.asarray(wi1, np.float32)
    wh1 = np.asarray(wh1, np.float32)

    # L0 center-pair elem1: lone x tap (2,1), K=64 padded to 128 with zeros
    s4 = np.zeros((CH, 4 * CH), np.float32)
    s4[:C, :] = wi0[:, :, 2, 1].T
    # L1 center-pair elem1: i2h center tap (K=128)
    w1c = wi1[:, :, 1, 1].T

    shared = {
        "w0": _prep_w0_pairs(wi0),
        "u0": np.concatenate([_prep_w_pairs(wh0),
                              _prep_center_pair(wh0, s4)], axis=1),
        "w1": _prep_w_pairs(wi1),
        "u1": np.concatenate([_prep_w_pairs(wh1),
                              _prep_center_pair(wh1, w1c)], axis=1),
        "wt": _prep_w_flat(np.asarray(wtop, np.float32)),
        "b0": np.ascontiguousarray(
            (np.asarray(bi0) + np.asarray(bh0)).astype(np.float32)
            .reshape(4, CH).T),
        "b1": np.ascontiguousarray(
            (np.asarray(bi1) + np.asarray(bh1)).astype(np.float32)
            .reshape(4, CH).T),
        "bt": np.ascontiguousarray(np.tile(
            np.asarray(btop, np.float32), 2).reshape(2 * C, 1)),
    }
    in_maps = []
    for b in range(B):
        m = dict(shared)
        m["xs"] = _cast_fp8(target[b, :NSTEP].reshape(NSTEP, C, HW))
        m["h0i"] = _cast_fp8(np.asarray(h0, np.float32)[b].reshape(CH, HW))
        m["c0i"] = np.ascontiguousarray(
            np.asarray(c0, np.float32)[b].reshape(CH, HW))
        h1b = np.asarray(h1, np.float32)[b].reshape(CH, HW)
        m["h1i8"] = _cast_fp8(h1b)
        m["h1i6"] = _cast_bf16(h1b)
        m["c1i"] = np.ascontiguousarray(
            np.asarray(c1, np.float32)[b].reshape(CH, HW))
        in_maps.append(m)

    res = run_bass_kernel_spmd(nc, in_maps, core_ids=list(range(B)))
    out = np.stack([res.results[b]["out"].reshape(T, C, H, W)
                    for b in range(B)])
    return out
